# revision 6
# baseline (speedup 1.0000x reference)
"""AttnDecoder kernel for 8 trn2 NeuronCores.

Math: the reference's additive attention has NO nonlinearity between W1/W2/w3,
so scores[b,t,s] = enc[b,s]@ue + dec[b,t]@ud + const. Softmax over s cancels
the t-dependent terms exactly -> attn (and ctx) are t-independent:
    ue  = W1[:, :H].T @ (W2.T @ w3[0])
    attn[b, :] = softmax(enc[b] @ ue);  ctx[b] = attn[b] @ enc[b]
Device work = 2-layer LSTM (replicated on all 8 cores, batch-in-M col-tiled
matmuls) + vocab-sharded output projection (4000 cols/core). No collectives.
"""

import numpy as np
import ml_dtypes

B, T, S = 8, 64, 128
V, E, H = 32000, 512, 512
NCORES = 8
VS = V // NCORES  # 4000 vocab columns per core
NSLICE = 500      # psum-bank-sized N chunk for the projection

_BF16 = ml_dtypes.bfloat16


def _reorder_w(Wih, Whh):
    """[128, 8*2048] rhs layout: rounds 0-3 = Wih K-chunks, 4-7 = Whh K-chunks.
    col j*512 + g*128 + x  <-  W[g*512 + 128j + x, 128*ki + p]; g-gate rows x2
    (tanh(z) = 2*sigmoid(2z) - 1 lets one Sigmoid call cover all gates)."""
    out = np.zeros((128, 8 * 2048), np.float32)
    for r in range(8):
        Wsrc = Wih if r < 4 else Whh
        ki = r % 4
        blk = Wsrc[:, 128 * ki:128 * (ki + 1)]          # [2048, 128] (gates, p)
        t_ = blk.reshape(4, 4, 128, 128)                # [g, j, x, p]
        t_ = t_.transpose(3, 1, 0, 2)                   # [p, j, g, x]
        out[:, r * 2048:(r + 1) * 2048] = t_.reshape(128, 2048)
    w5 = out.reshape(128, 8, 4, 4, 128)                 # [p, r, j, g, x]
    w5[:, :, :, 2, :] *= 2.0
    return out


def _reorder_b(bih, bhh):
    bb = (bih + bhh).astype(np.float32).reshape(4, 4, 128)  # [g, j, x]
    bb = bb.transpose(1, 0, 2).copy()                       # [j, g, x]
    bb[:, 2, :] *= 2.0
    return bb.reshape(1, 2048)


def _build_nc():
    import concourse.bass as bass
    import concourse.bacc as bacc
    import concourse.mybir as mybir
    import concourse.tile as tile

    f32 = mybir.dt.float32
    bf16 = mybir.dt.bfloat16
    AF = mybir.ActivationFunctionType
    OP = mybir.AluOpType

    nc = bacc.Bacc(None, target_bir_lowering=False)
    d = {}
    d["xT"] = nc.dram_tensor("xT", [128, 4 * 512], bf16, kind="ExternalInput")
    d["h0a"] = nc.dram_tensor("h0a", [128, 32], bf16, kind="ExternalInput")
    d["h0b"] = nc.dram_tensor("h0b", [128, 32], bf16, kind="ExternalInput")
    d["c0"] = nc.dram_tensor("c0", [128, 256], f32, kind="ExternalInput")
    d["W0"] = nc.dram_tensor("W0", [128, 8 * 2048], bf16, kind="ExternalInput")
    d["W1"] = nc.dram_tensor("W1", [128, 8 * 2048], bf16, kind="ExternalInput")
    d["b0"] = nc.dram_tensor("b0", [1, 2048], bf16, kind="ExternalInput")
    d["b1"] = nc.dram_tensor("b1", [1, 2048], bf16, kind="ExternalInput")
    d["Wo"] = nc.dram_tensor("Wo", [128, 8 * VS], bf16, kind="ExternalInput")
    d["bo"] = nc.dram_tensor("bo", [1, VS], bf16, kind="ExternalInput")
    d["ctxT"] = nc.dram_tensor("ctxT", [128, 4 * 128], bf16, kind="ExternalInput")
    d["id8"] = nc.dram_tensor("id8", [8, 8], f32, kind="ExternalInput")
    d["ones"] = nc.dram_tensor("ones", [1, 128], bf16, kind="ExternalInput")
    out_d = nc.dram_tensor("out", [512, VS], f32, kind="ExternalOutput")

    with tile.TileContext(nc) as tc:
        with (
            tc.tile_pool(name="const", bufs=1) as cp,
            tc.tile_pool(name="work", bufs=2) as wp,
            tc.tile_pool(name="ps", bufs=1, space="PSUM") as pp,
            tc.tile_pool(name="ps2", bufs=2, space="PSUM") as pp2,
        ):
            W0s = cp.tile([128, 8 * 2048], bf16, tag="W0s")
            W1s = cp.tile([128, 8 * 2048], bf16, tag="W1s")
            Wos = cp.tile([128, 8 * VS], bf16, tag="Wos")
            xTs = cp.tile([128, 4 * 512], bf16, tag="xTs")
            dec0T = cp.tile([128, 65 * 32], bf16, tag="dec0T")
            decT = cp.tile([128, 65 * 32], bf16, tag="decT")
            c_sb = cp.tile([128, 256], f32, tag="c_sb")
            b0s = cp.tile([1, 2048], bf16, tag="b0s")
            b1s = cp.tile([1, 2048], bf16, tag="b1s")
            bos = cp.tile([1, VS], bf16, tag="bos")
            ctxs = cp.tile([128, 4 * 128], bf16, tag="ctxs")
            id8s = cp.tile([8, 8], f32, tag="id8s")
            ones = cp.tile([1, 128], bf16, tag="ones")

            # L0 weights first (needed immediately), big tensors chunked
            for r in range(8):
                nc.sync.dma_start(W0s[:, r * 2048:(r + 1) * 2048],
                                  d["W0"][:, r * 2048:(r + 1) * 2048])
            nc.sync.dma_start(xTs[:], d["xT"][:])
            nc.sync.dma_start(dec0T[:, 0:32], d["h0a"][:])
            nc.sync.dma_start(decT[:, 0:32], d["h0b"][:])
            nc.sync.dma_start(c_sb[:], d["c0"][:])
            nc.sync.dma_start(b0s[:], d["b0"][:])
            nc.sync.dma_start(b1s[:], d["b1"][:])
            nc.sync.dma_start(id8s[:], d["id8"][:])
            nc.sync.dma_start(ones[:], d["ones"][:])
            for r in range(8):
                nc.sync.dma_start(W1s[:, r * 2048:(r + 1) * 2048],
                                  d["W1"][:, r * 2048:(r + 1) * 2048])
            for r in range(8):
                nc.sync.dma_start(Wos[:, r * VS:(r + 1) * VS],
                                  d["Wo"][:, r * VS:(r + 1) * VS])
            nc.sync.dma_start(bos[:], d["bo"][:])
            nc.sync.dma_start(ctxs[:], d["ctxT"][:])

            psg = [pp.tile([128, 512], f32, tag="psg0", name="psg0"),
                   pp.tile([128, 512], f32, tag="psg1", name="psg1")]
            psT = [pp.tile([128, 32], f32, tag="psT0", name="psT0"),
                   pp.tile([128, 32], f32, tag="psT1", name="psT1")]

            Ws = [W0s, W1s]
            bs = [b0s, b1s]
            own = [dec0T, decT]

            def lstm_step(layer, t):
                ps = psg[layer]
                pT = psT[layer]
                for r in range(9):
                    if r < 4:
                        if layer == 0:
                            lhs = xTs[:, r * 512 + 8 * t: r * 512 + 8 * t + 8]
                        else:
                            lhs = dec0T[:, 32 * (t + 1) + 8 * r:
                                        32 * (t + 1) + 8 * r + 8]
                    elif r < 8:
                        k = r - 4
                        lhs = own[layer][:, 32 * t + 8 * k: 32 * t + 8 * k + 8]
                    else:
                        lhs = ones[0:1, 0:8]
                    for j in range(4):
                        if r < 8:
                            rhs = Ws[layer][:, r * 2048 + j * 512:
                                            r * 2048 + (j + 1) * 512]
                        else:
                            rhs = bs[layer][0:1, j * 512:(j + 1) * 512]
                        nc.tensor.matmul(
                            ps[32 * j:32 * j + 8, :], lhs, rhs,
                            start=(r == 0), stop=(r == 8),
                            tile_position=(0, 32 * j))
                sg = wp.tile([128, 512], f32, tag=f"sg{layer}")
                nc.scalar.activation(sg[0:104, :], ps[0:104, :], AF.Sigmoid)
                cs = c_sb[0:104, layer * 128:(layer + 1) * 128]
                tg = wp.tile([128, 128], f32, tag=f"tg{layer}")
                nc.vector.tensor_scalar(tg[0:104, :], sg[0:104, 256:384],
                                        2.0, -1.0, OP.mult, OP.add)
                m2 = wp.tile([128, 128], f32, tag=f"m2{layer}")
                nc.vector.tensor_mul(m2[0:104, :], sg[0:104, 0:128], tg[0:104, :])
                m1 = wp.tile([128, 128], f32, tag=f"m1{layer}")
                nc.vector.tensor_mul(m1[0:104, :], sg[0:104, 128:256], cs)
                nc.vector.tensor_add(cs, m1[0:104, :], m2[0:104, :])
                sc2 = wp.tile([128, 128], f32, tag=f"sc{layer}")
                nc.scalar.activation(sc2[0:104, :], cs, AF.Sigmoid, scale=2.0)
                tcc = wp.tile([128, 128], f32, tag=f"tc{layer}")
                nc.vector.tensor_scalar(tcc[0:104, :], sc2[0:104, :],
                                        2.0, -1.0, OP.mult, OP.add)
                hsp = wp.tile([128, 128], f32, tag=f"hs{layer}")
                nc.vector.tensor_mul(hsp[0:104, :], sg[0:104, 384:512],
                                     tcc[0:104, :])
                hfl = wp.tile([8, 512], f32, tag=f"hf{layer}")
                for j in range(4):
                    nc.vector.tensor_copy(hfl[0:8, 128 * j:128 * (j + 1)],
                                          hsp[32 * j:32 * j + 8, 0:128])
                for k in range(4):
                    nc.tensor.transpose(pT[:, 8 * k:8 * k + 8],
                                        hfl[0:8, 128 * k:128 * (k + 1)],
                                        id8s[:])
                nc.vector.tensor_copy(
                    own[layer][:, 32 * (t + 1):32 * (t + 2)], pT[:, 0:32])

            for t in range(T):
                lstm_step(0, t)
                if t >= 1:
                    lstm_step(1, t - 1)
            lstm_step(1, T - 1)

            decv = decT.rearrange("p (s c) -> p s c", c=32)
            projT = cp.tile([128, 4 * 512], bf16, tag="projT")
            for m in range(4):
                for r in range(4):
                    nc.vector.tensor_copy(
                        projT[:, m * 512 + r * 128: m * 512 + (r + 1) * 128],
                        decv[:, 1 + 16 * m: 17 + 16 * m, 8 * r: 8 * r + 8])
            for m in range(4):
                for n in range(8):
                    pso = pp2.tile([128, NSLICE], f32, tag="po")
                    for r in range(9):
                        if r < 4:
                            lhs = projT[:, m * 512 + r * 128:
                                        m * 512 + (r + 1) * 128]
                        elif r < 8:
                            lhs = ctxs[:, (r - 4) * 128:(r - 3) * 128]
                        else:
                            lhs = ones[0:1, 0:128]
                        if r < 8:
                            rhs = Wos[:, r * VS + n * NSLICE:
                                      r * VS + (n + 1) * NSLICE]
                        else:
                            rhs = bos[0:1, n * NSLICE:(n + 1) * NSLICE]
                        nc.tensor.matmul(pso[:, :], lhs, rhs,
                                         start=(r == 0), stop=(r == 8))
                    ob = wp.tile([128, NSLICE], f32, tag="ob")
                    nc.vector.tensor_copy(ob[:], pso[:])
                    nc.sync.dma_start(
                        out_d[128 * m:128 * (m + 1),
                              n * NSLICE:(n + 1) * NSLICE], ob[:])
    nc.finalize()
    return nc


_NC_CACHE = None


def _get_nc():
    global _NC_CACHE
    if _NC_CACHE is None:
        _NC_CACHE = _build_nc()
    return _NC_CACHE


def _host_inputs(input_ids, enc_output, h0, c0, emb, Wih0, Whh0, bih0, bhh0,
                 Wih1, Whh1, bih1, bhh1, W1, b1, W2, b2, w3, b3, Wout, bout):
    f32 = np.float32
    x = np.asarray(emb, f32)[np.asarray(input_ids).astype(np.int64)]  # [B,T,E]
    xr = x.transpose(2, 1, 0).reshape(4, 128, T, B)       # [e,p,t,b]
    xT = xr.transpose(1, 0, 2, 3).reshape(128, 4 * 512)

    def h0T(hl):
        return hl.T.reshape(4, 128, 8).transpose(1, 0, 2).reshape(128, 32)

    c_sp = np.zeros((128, 256), f32)
    for layer in range(2):
        for j in range(4):
            c_sp[32 * j:32 * j + 8, layer * 128:(layer + 1) * 128] = \
                np.asarray(c0, f32)[layer, :, 128 * j:128 * (j + 1)]

    # collapsed attention (exact in real arithmetic; see module docstring)
    u = np.asarray(W2, f32).T @ np.asarray(w3, f32)[0]
    ue = np.asarray(W1, f32)[:, :H].T @ u
    sc = np.asarray(enc_output, f32) @ ue                  # [B,S]
    sc = sc - sc.max(-1, keepdims=True)
    a = np.exp(sc)
    a /= a.sum(-1, keepdims=True)
    ctxh = np.einsum('bs,bsh->bh', a, np.asarray(enc_output, f32))  # [B,H]
    cT = ctxh.T.reshape(4, 128, 8)                         # [k,p,b]
    ctxT = np.concatenate(
        [np.tile(cT[k][:, None, :], (1, 16, 1)).reshape(128, 128)
         for k in range(4)], axis=1)                       # [128, 512]

    base = {
        "xT": xT.astype(_BF16),
        "h0a": h0T(np.asarray(h0, f32)[0]).astype(_BF16),
        "h0b": h0T(np.asarray(h0, f32)[1]).astype(_BF16),
        "c0": c_sp,
        "W0": _reorder_w(np.asarray(Wih0, f32), np.asarray(Whh0, f32)).astype(_BF16),
        "W1": _reorder_w(np.asarray(Wih1, f32), np.asarray(Whh1, f32)).astype(_BF16),
        "b0": _reorder_b(np.asarray(bih0, f32), np.asarray(bhh0, f32)).astype(_BF16),
        "b1": _reorder_b(np.asarray(bih1, f32), np.asarray(bhh1, f32)).astype(_BF16),
        "ctxT": ctxT.astype(_BF16),
        "id8": np.eye(8, dtype=f32),
        "ones": np.ones((1, 128), f32).astype(_BF16),
    }
    Wo_full = np.asarray(Wout, f32)                        # [V, 2H]
    bo_full = np.asarray(bout, f32)
    maps = []
    for k in range(NCORES):
        sh = Wo_full[k * VS:(k + 1) * VS]                  # [VS, 1024]
        Wo = np.concatenate([sh[:, 128 * r:128 * (r + 1)].T
                             for r in range(8)], axis=1)   # [128, 8*VS]
        m = dict(base)
        m["Wo"] = np.ascontiguousarray(Wo).astype(_BF16)
        m["bo"] = bo_full[k * VS:(k + 1) * VS].reshape(1, VS).astype(_BF16)
        maps.append(m)
    return maps


def kernel(**inputs):
    from concourse.bass_utils import run_bass_kernel_spmd
    nc = _get_nc()
    maps = _host_inputs(**inputs)
    res = run_bass_kernel_spmd(nc, maps, list(range(NCORES))).results
    full = np.zeros((B, T, V), np.float32)
    for k in range(NCORES):
        o = np.asarray(res[k]["out"], np.float32).reshape(T, B, VS)
        full[:, :, k * VS:(k + 1) * VS] = o.transpose(1, 0, 2)
    return full


# revision 8
# speedup vs baseline: 4339.7223x; 4339.7223x over previous
"""AttnDecoder kernel for 8 trn2 NeuronCores.

Math: the reference's additive attention has NO nonlinearity between W1/W2/w3,
so scores[b,t,s] = enc[b,s]@ue + dec[b,t]@ud + const. Softmax over s cancels
the t-dependent terms exactly -> attn (and ctx) are t-independent:
    ue  = W1[:, :H].T @ (W2.T @ w3[0])
    attn[b, :] = softmax(enc[b] @ ue);  ctx[b] = attn[b] @ enc[b]
Device work = 2-layer LSTM (replicated on all 8 cores, batch-in-M col-tiled
matmuls) + vocab-sharded output projection (4000 cols/core). No collectives.
"""

import numpy as np
import ml_dtypes

B, T, S = 8, 64, 128
V, E, H = 32000, 512, 512
NCORES = 8
VS = V // NCORES  # 4000 vocab columns per core
NSLICE = 500      # psum-bank-sized N chunk for the projection

_BF16 = ml_dtypes.bfloat16


def _reorder_w(Wih, Whh):
    """[128, 8*2048] rhs layout: rounds 0-3 = Wih K-chunks, 4-7 = Whh K-chunks.
    col j*512 + g*128 + x  <-  W[g*512 + 128j + x, 128*ki + p]; g-gate rows x2
    (tanh(z) = 2*sigmoid(2z) - 1 lets one Sigmoid call cover all gates)."""
    out = np.zeros((128, 8 * 2048), np.float32)
    for r in range(8):
        Wsrc = Wih if r < 4 else Whh
        ki = r % 4
        blk = Wsrc[:, 128 * ki:128 * (ki + 1)]          # [2048, 128] (gates, p)
        t_ = blk.reshape(4, 4, 128, 128)                # [g, j, x, p]
        t_ = t_.transpose(3, 1, 0, 2)                   # [p, j, g, x]
        out[:, r * 2048:(r + 1) * 2048] = t_.reshape(128, 2048)
    w5 = out.reshape(128, 8, 4, 4, 128)                 # [p, r, j, g, x]
    w5[:, :, :, 2, :] *= 2.0
    return out


def _id104():
    m = np.zeros((104, 8), np.float32)
    for j in range(4):
        m[32 * j:32 * j + 8, :] = np.eye(8, dtype=np.float32)
    return m


def _reorder_b(bih, bhh):
    bb = (bih + bhh).astype(np.float32).reshape(4, 4, 128)  # [g, j, x]
    bb = bb.transpose(1, 0, 2).copy()                       # [j, g, x]
    bb[:, 2, :] *= 2.0
    return bb.reshape(1, 2048)


def _build_nc():
    import concourse.bass as bass
    import concourse.bacc as bacc
    import concourse.mybir as mybir
    import concourse.tile as tile

    f32 = mybir.dt.float32
    bf16 = mybir.dt.bfloat16
    AF = mybir.ActivationFunctionType
    OP = mybir.AluOpType

    nc = bacc.Bacc(None, target_bir_lowering=False)
    d = {}
    d["xT"] = nc.dram_tensor("xT", [128, 4 * 512], bf16, kind="ExternalInput")
    d["h0a"] = nc.dram_tensor("h0a", [128, 32], bf16, kind="ExternalInput")
    d["h0b"] = nc.dram_tensor("h0b", [128, 32], bf16, kind="ExternalInput")
    d["c0"] = nc.dram_tensor("c0", [128, 256], f32, kind="ExternalInput")
    d["W0"] = nc.dram_tensor("W0", [128, 8 * 2048], bf16, kind="ExternalInput")
    d["W1"] = nc.dram_tensor("W1", [128, 8 * 2048], bf16, kind="ExternalInput")
    d["b0"] = nc.dram_tensor("b0", [1, 2048], bf16, kind="ExternalInput")
    d["b1"] = nc.dram_tensor("b1", [1, 2048], bf16, kind="ExternalInput")
    d["Wo"] = nc.dram_tensor("Wo", [128, 8 * VS], bf16, kind="ExternalInput")
    d["bo"] = nc.dram_tensor("bo", [1, VS], bf16, kind="ExternalInput")
    d["ctxT"] = nc.dram_tensor("ctxT", [128, 4 * 128], bf16, kind="ExternalInput")
    d["id8"] = nc.dram_tensor("id8", [104, 8], f32, kind="ExternalInput")
    d["ones"] = nc.dram_tensor("ones", [1, 128], bf16, kind="ExternalInput")
    out_d = nc.dram_tensor("out", [512, VS], f32, kind="ExternalOutput")

    with tile.TileContext(nc) as tc:
        with (
            tc.tile_pool(name="const", bufs=1) as cp,
            tc.tile_pool(name="work", bufs=2) as wp,
            tc.tile_pool(name="ps", bufs=1, space="PSUM") as pp,
            tc.tile_pool(name="psg", bufs=2, space="PSUM") as ppg,
            tc.tile_pool(name="ps2", bufs=2, space="PSUM") as pp2,
        ):
            W0s = cp.tile([128, 8 * 2048], bf16, tag="W0s")
            W1s = cp.tile([128, 8 * 2048], bf16, tag="W1s")
            Wos = cp.tile([128, 8 * VS], bf16, tag="Wos")
            xTs = cp.tile([128, 4 * 512], bf16, tag="xTs")
            dec0T = cp.tile([128, 65 * 32], bf16, tag="dec0T")
            decT = cp.tile([128, 65 * 32], bf16, tag="decT")
            c_sb = cp.tile([128, 256], f32, tag="c_sb")
            b0s = cp.tile([1, 2048], bf16, tag="b0s")
            b1s = cp.tile([1, 2048], bf16, tag="b1s")
            bos = cp.tile([1, VS], bf16, tag="bos")
            ctxs = cp.tile([128, 4 * 128], bf16, tag="ctxs")
            id8s = cp.tile([104, 8], f32, tag="id8s")
            ones = cp.tile([1, 128], bf16, tag="ones")

            # L0 weights first (needed immediately), big tensors chunked
            for r in range(8):
                nc.sync.dma_start(W0s[:, r * 2048:(r + 1) * 2048],
                                  d["W0"][:, r * 2048:(r + 1) * 2048])
            nc.sync.dma_start(xTs[:], d["xT"][:])
            nc.sync.dma_start(dec0T[:, 0:32], d["h0a"][:])
            nc.sync.dma_start(decT[:, 0:32], d["h0b"][:])
            nc.sync.dma_start(c_sb[:], d["c0"][:])
            nc.sync.dma_start(b0s[:], d["b0"][:])
            nc.sync.dma_start(b1s[:], d["b1"][:])
            nc.sync.dma_start(id8s[:], d["id8"][:])
            nc.sync.dma_start(ones[:], d["ones"][:])
            for r in range(8):
                nc.sync.dma_start(W1s[:, r * 2048:(r + 1) * 2048],
                                  d["W1"][:, r * 2048:(r + 1) * 2048])
            for r in range(8):
                nc.sync.dma_start(Wos[:, r * VS:(r + 1) * VS],
                                  d["Wo"][:, r * VS:(r + 1) * VS])
            nc.sync.dma_start(bos[:], d["bo"][:])
            nc.sync.dma_start(ctxs[:], d["ctxT"][:])

            psT = [pp.tile([128, 32], f32, tag="psT0", name="psT0"),
                   pp.tile([128, 32], f32, tag="psT1", name="psT1")]

            Ws = [W0s, W1s]
            bs = [b0s, b1s]
            own = [dec0T, decT]

            def lstm_step(layer, t):
                ps = ppg.tile([128, 512], f32, tag=f"psg{layer}",
                              name=f"psg{layer}_{t}")
                pT = psT[layer]
                for r in range(9):
                    if r < 4:
                        if layer == 0:
                            lhs = xTs[:, r * 512 + 8 * t: r * 512 + 8 * t + 8]
                        else:
                            lhs = dec0T[:, 32 * (t + 1) + 8 * r:
                                        32 * (t + 1) + 8 * r + 8]
                    elif r < 8:
                        k = r - 4
                        lhs = own[layer][:, 32 * t + 8 * k: 32 * t + 8 * k + 8]
                    else:
                        lhs = ones[0:1, 0:8]
                    for j in range(4):
                        if r < 8:
                            rhs = Ws[layer][:, r * 2048 + j * 512:
                                            r * 2048 + (j + 1) * 512]
                        else:
                            rhs = bs[layer][0:1, j * 512:(j + 1) * 512]
                        nc.tensor.matmul(
                            ps[32 * j:32 * j + 8, :], lhs, rhs,
                            start=(r == 0), stop=(r == 8),
                            tile_position=(0, 32 * j))
                sg = wp.tile([128, 512], f32, tag=f"sg{layer}")
                nc.scalar.activation(sg[0:104, :], ps[0:104, :], AF.Sigmoid)
                cs = c_sb[0:104, layer * 128:(layer + 1) * 128]
                tg = wp.tile([128, 128], f32, tag=f"tg{layer}")
                nc.vector.tensor_scalar(tg[0:104, :], sg[0:104, 256:384],
                                        2.0, -1.0, OP.mult, OP.add)
                m2 = wp.tile([128, 128], f32, tag=f"m2{layer}")
                nc.vector.tensor_mul(m2[0:104, :], sg[0:104, 0:128], tg[0:104, :])
                m1 = wp.tile([128, 128], f32, tag=f"m1{layer}")
                nc.vector.tensor_mul(m1[0:104, :], sg[0:104, 128:256], cs)
                nc.vector.tensor_add(cs, m1[0:104, :], m2[0:104, :])
                sc2 = wp.tile([128, 128], f32, tag=f"sc{layer}")
                nc.scalar.activation(sc2[0:104, :], cs, AF.Sigmoid, scale=2.0)
                tcc = wp.tile([128, 128], f32, tag=f"tc{layer}")
                nc.vector.tensor_scalar(tcc[0:104, :], sc2[0:104, :],
                                        2.0, -1.0, OP.mult, OP.add)
                hsp = wp.tile([128, 128], f32, tag=f"hs{layer}")
                nc.vector.tensor_mul(hsp[0:104, :], sg[0:104, 384:512],
                                     tcc[0:104, :])
                hfl = wp.tile([8, 512], f32, tag=f"hf{layer}")
                for j in range(4):
                    nc.vector.tensor_copy(hfl[0:8, 128 * j:128 * (j + 1)],
                                          hsp[32 * j:32 * j + 8, 0:128])
                for k in range(4):
                    nc.tensor.transpose(pT[:, 8 * k:8 * k + 8],
                                        hfl[0:8, 128 * k:128 * (k + 1)],
                                        id8s[0:8, :])
                nc.vector.tensor_copy(
                    own[layer][:, 32 * (t + 1):32 * (t + 2)], pT[:, 0:32])

            for t in range(T):
                lstm_step(0, t)
                if t >= 1:
                    lstm_step(1, t - 1)
            lstm_step(1, T - 1)

            decv = decT.rearrange("p (s c) -> p s c", c=32)
            projT = cp.tile([128, 4 * 512], bf16, tag="projT")
            for m in range(4):
                for r in range(4):
                    nc.vector.tensor_copy(
                        projT[:, m * 512 + r * 128: m * 512 + (r + 1) * 128],
                        decv[:, 1 + 16 * m: 17 + 16 * m, 8 * r: 8 * r + 8])
            for m in range(4):
                for n in range(8):
                    pso = pp2.tile([128, NSLICE], f32, tag="po")
                    for r in range(9):
                        if r < 4:
                            lhs = projT[:, m * 512 + r * 128:
                                        m * 512 + (r + 1) * 128]
                        elif r < 8:
                            lhs = ctxs[:, (r - 4) * 128:(r - 3) * 128]
                        else:
                            lhs = ones[0:1, 0:128]
                        if r < 8:
                            rhs = Wos[:, r * VS + n * NSLICE:
                                      r * VS + (n + 1) * NSLICE]
                        else:
                            rhs = bos[0:1, n * NSLICE:(n + 1) * NSLICE]
                        nc.tensor.matmul(pso[:, :], lhs, rhs,
                                         start=(r == 0), stop=(r == 8))
                    ob = wp.tile([128, NSLICE], f32, tag="ob")
                    nc.vector.tensor_copy(ob[:], pso[:])
                    nc.sync.dma_start(
                        out_d[128 * m:128 * (m + 1),
                              n * NSLICE:(n + 1) * NSLICE], ob[:])
    nc.finalize()
    return nc


_NC_CACHE = None


def _get_nc():
    global _NC_CACHE
    if _NC_CACHE is None:
        _NC_CACHE = _build_nc()
    return _NC_CACHE


def _host_inputs(input_ids, enc_output, h0, c0, emb, Wih0, Whh0, bih0, bhh0,
                 Wih1, Whh1, bih1, bhh1, W1, b1, W2, b2, w3, b3, Wout, bout):
    f32 = np.float32
    x = np.asarray(emb, f32)[np.asarray(input_ids).astype(np.int64)]  # [B,T,E]
    xr = x.transpose(2, 1, 0).reshape(4, 128, T, B)       # [e,p,t,b]
    xT = xr.transpose(1, 0, 2, 3).reshape(128, 4 * 512)

    def h0T(hl):
        return hl.T.reshape(4, 128, 8).transpose(1, 0, 2).reshape(128, 32)

    c_sp = np.zeros((128, 256), f32)
    for layer in range(2):
        for j in range(4):
            c_sp[32 * j:32 * j + 8, layer * 128:(layer + 1) * 128] = \
                np.asarray(c0, f32)[layer, :, 128 * j:128 * (j + 1)]

    # collapsed attention (exact in real arithmetic; see module docstring)
    u = np.asarray(W2, f32).T @ np.asarray(w3, f32)[0]
    ue = np.asarray(W1, f32)[:, :H].T @ u
    sc = np.asarray(enc_output, f32) @ ue                  # [B,S]
    sc = sc - sc.max(-1, keepdims=True)
    a = np.exp(sc)
    a /= a.sum(-1, keepdims=True)
    ctxh = np.einsum('bs,bsh->bh', a, np.asarray(enc_output, f32))  # [B,H]
    cT = ctxh.T.reshape(4, 128, 8)                         # [k,p,b]
    ctxT = np.concatenate(
        [np.tile(cT[k][:, None, :], (1, 16, 1)).reshape(128, 128)
         for k in range(4)], axis=1)                       # [128, 512]

    base = {
        "xT": xT.astype(_BF16),
        "h0a": h0T(np.asarray(h0, f32)[0]).astype(_BF16),
        "h0b": h0T(np.asarray(h0, f32)[1]).astype(_BF16),
        "c0": c_sp,
        "W0": _reorder_w(np.asarray(Wih0, f32), np.asarray(Whh0, f32)).astype(_BF16),
        "W1": _reorder_w(np.asarray(Wih1, f32), np.asarray(Whh1, f32)).astype(_BF16),
        "b0": _reorder_b(np.asarray(bih0, f32), np.asarray(bhh0, f32)).astype(_BF16),
        "b1": _reorder_b(np.asarray(bih1, f32), np.asarray(bhh1, f32)).astype(_BF16),
        "ctxT": ctxT.astype(_BF16),
        "id8": _id104(),
        "ones": np.ones((1, 128), f32).astype(_BF16),
    }
    Wo_full = np.asarray(Wout, f32)                        # [V, 2H]
    bo_full = np.asarray(bout, f32)
    maps = []
    for k in range(NCORES):
        sh = Wo_full[k * VS:(k + 1) * VS]                  # [VS, 1024]
        Wo = np.concatenate([sh[:, 128 * r:128 * (r + 1)].T
                             for r in range(8)], axis=1)   # [128, 8*VS]
        m = dict(base)
        m["Wo"] = np.ascontiguousarray(Wo).astype(_BF16)
        m["bo"] = bo_full[k * VS:(k + 1) * VS].reshape(1, VS).astype(_BF16)
        maps.append(m)
    return maps


def kernel(**inputs):
    from concourse.bass_utils import run_bass_kernel_spmd
    nc = _get_nc()
    maps = _host_inputs(**inputs)
    res = run_bass_kernel_spmd(nc, maps, list(range(NCORES))).results
    full = np.zeros((B, T, V), np.float32)
    for k in range(NCORES):
        o = np.asarray(res[k]["out"], np.float32).reshape(T, B, VS)
        full[:, :, k * VS:(k + 1) * VS] = o.transpose(1, 0, 2)
    return full


# revision 14
# speedup vs baseline: 5078.5294x; 1.1702x over previous
"""AttnDecoder kernel for 8 trn2 NeuronCores.

Math: the reference's additive attention has NO nonlinearity between W1/W2/w3,
so scores[b,t,s] = enc[b,s]@ue + dec[b,t]@ud + const. Softmax over s cancels
the t-dependent terms exactly -> attn (and ctx) are t-independent:
    ue  = W1[:, :H].T @ (W2.T @ w3[0])
    attn[b, :] = softmax(enc[b] @ ue);  ctx[b] = attn[b] @ enc[b]
Device work = 2-layer LSTM (replicated on all 8 cores, batch-in-M col-tiled
matmuls) + vocab-sharded output projection (4000 cols/core). No collectives.
"""

import numpy as np
import ml_dtypes

B, T, S = 8, 64, 128
V, E, H = 32000, 512, 512
NCORES = 8
VS = V // NCORES  # 4000 vocab columns per core
NSLICE = 500      # psum-bank-sized N chunk for the projection

_BF16 = ml_dtypes.bfloat16


def _reorder_w(Wih, Whh):
    """[128, 8*2048] rhs layout: rounds 0-3 = Wih K-chunks, 4-7 = Whh K-chunks.
    col j*512 + g*128 + x  <-  W[g*512 + 128j + x, 128*ki + p]; g-gate rows x2
    (tanh(z) = 2*sigmoid(2z) - 1 lets one Sigmoid call cover all gates)."""
    out = np.zeros((128, 8 * 2048), np.float32)
    for r in range(8):
        Wsrc = Wih if r < 4 else Whh
        ki = r % 4
        blk = Wsrc[:, 128 * ki:128 * (ki + 1)]          # [2048, 128] (gates, p)
        t_ = blk.reshape(4, 4, 128, 128)                # [g, j, x, p]
        t_ = t_.transpose(3, 1, 0, 2)                   # [p, j, g, x]
        out[:, r * 2048:(r + 1) * 2048] = t_.reshape(128, 2048)
    w5 = out.reshape(128, 8, 4, 4, 128)                 # [p, r, j, g, x]
    w5[:, :, :, 2, :] *= 2.0
    return out


def _sel32():
    m = np.zeros((32, 32), np.float32)
    for q in range(4):
        m[8 * q:8 * (q + 1), 8 * q:8 * (q + 1)] = np.eye(8, dtype=np.float32)
    return m


def _id104():
    m = np.zeros((104, 8), np.float32)
    for j in range(4):
        m[32 * j:32 * j + 8, :] = np.eye(8, dtype=np.float32)
    return m


def _reorder_b(bih, bhh):
    bb = (bih + bhh).astype(np.float32).reshape(4, 4, 128)  # [g, j, x]
    bb = bb.transpose(1, 0, 2).copy()                       # [j, g, x]
    bb[:, 2, :] *= 2.0
    return bb.reshape(1, 2048)


def _build_nc():
    import concourse.bass as bass
    import concourse.bacc as bacc
    import concourse.mybir as mybir
    import concourse.tile as tile

    f32 = mybir.dt.float32
    bf16 = mybir.dt.bfloat16
    AF = mybir.ActivationFunctionType
    OP = mybir.AluOpType

    nc = bacc.Bacc(None, target_bir_lowering=False)
    d = {}
    d["xT"] = nc.dram_tensor("xT", [128, 4 * 512], bf16, kind="ExternalInput")
    d["h0a"] = nc.dram_tensor("h0a", [128, 32], bf16, kind="ExternalInput")
    d["h0b"] = nc.dram_tensor("h0b", [128, 32], bf16, kind="ExternalInput")
    d["c0"] = nc.dram_tensor("c0", [128, 256], f32, kind="ExternalInput")
    d["W0"] = nc.dram_tensor("W0", [128, 8 * 2048], bf16, kind="ExternalInput")
    d["W1"] = nc.dram_tensor("W1", [128, 8 * 2048], bf16, kind="ExternalInput")
    d["b0"] = nc.dram_tensor("b0", [1, 2048], bf16, kind="ExternalInput")
    d["b1"] = nc.dram_tensor("b1", [1, 2048], bf16, kind="ExternalInput")
    d["Wo"] = nc.dram_tensor("Wo", [128, 8 * VS], bf16, kind="ExternalInput")
    d["bo"] = nc.dram_tensor("bo", [1, VS], bf16, kind="ExternalInput")
    d["ctxT"] = nc.dram_tensor("ctxT", [128, 4 * 128], bf16, kind="ExternalInput")
    d["id8"] = nc.dram_tensor("id8", [104, 8], f32, kind="ExternalInput")
    d["ones"] = nc.dram_tensor("ones", [1, 128], bf16, kind="ExternalInput")
    d["id8b"] = nc.dram_tensor("id8b", [128, 32], bf16, kind="ExternalInput")
    out_d = nc.dram_tensor("out", [512, VS], f32, kind="ExternalOutput")

    with tile.TileContext(nc) as tc:
        with (
            tc.tile_pool(name="const", bufs=1) as cp,
            tc.tile_pool(name="work", bufs=2) as wp,
            tc.tile_pool(name="ps", bufs=1, space="PSUM") as pp,
            tc.tile_pool(name="psg", bufs=2, space="PSUM") as ppg,
            tc.tile_pool(name="ps2", bufs=2, space="PSUM") as pp2,
        ):
            W0s = cp.tile([128, 8 * 2048], bf16, tag="W0s")
            W1s = cp.tile([128, 8 * 2048], bf16, tag="W1s")
            Wos = cp.tile([128, 8 * VS], bf16, tag="Wos")
            xTs = cp.tile([128, 4 * 512], bf16, tag="xTs")
            dec0T = cp.tile([128, 65 * 32], bf16, tag="dec0T")
            decT = cp.tile([128, 65 * 32], bf16, tag="decT")
            c_sb = cp.tile([128, 256], f32, tag="c_sb")
            b0s = cp.tile([1, 2048], bf16, tag="b0s")
            b1s = cp.tile([1, 2048], bf16, tag="b1s")
            bos = cp.tile([1, VS], bf16, tag="bos")
            ctxs = cp.tile([128, 4 * 128], bf16, tag="ctxs")
            id8s = cp.tile([104, 8], f32, tag="id8s")
            ones = cp.tile([1, 128], bf16, tag="ones")
            id8b = cp.tile([128, 32], bf16, tag="id8b")

            # L0 weights first (needed immediately), big tensors chunked
            for r in range(8):
                nc.sync.dma_start(W0s[:, r * 2048:(r + 1) * 2048],
                                  d["W0"][:, r * 2048:(r + 1) * 2048])
            nc.sync.dma_start(xTs[:], d["xT"][:])
            nc.sync.dma_start(dec0T[:, 0:32], d["h0a"][:])
            nc.sync.dma_start(decT[:, 0:32], d["h0b"][:])
            nc.sync.dma_start(c_sb[:], d["c0"][:])
            nc.sync.dma_start(b0s[:], d["b0"][:])
            nc.sync.dma_start(b1s[:], d["b1"][:])
            nc.sync.dma_start(id8s[:], d["id8"][:])
            nc.sync.dma_start(ones[:], d["ones"][:])
            nc.sync.dma_start(id8b[:], d["id8b"][:])
            for r in range(8):
                nc.sync.dma_start(W1s[:, r * 2048:(r + 1) * 2048],
                                  d["W1"][:, r * 2048:(r + 1) * 2048])
            for r in range(8):
                nc.sync.dma_start(Wos[:, r * VS:(r + 1) * VS],
                                  d["Wo"][:, r * VS:(r + 1) * VS])
            nc.sync.dma_start(bos[:], d["bo"][:])
            nc.sync.dma_start(ctxs[:], d["ctxT"][:])

            psT = [pp.tile([128, 32], f32, tag="psT0", name="psT0"),
                   pp.tile([128, 32], f32, tag="psT1", name="psT1")]

            ig_sb = cp.tile([128, 4 * 2048], bf16, tag="ig_sb")
            for mt in range(4):
                for j in range(4):
                    pig = pp2.tile([128, 512], f32, tag="po", name=f"pig{mt}_{j}")
                    for ki in range(4):
                        nc.tensor.matmul(
                            pig[:, :],
                            xTs[:, ki * 512 + mt * 128: ki * 512 + (mt + 1) * 128],
                            W0s[:, ki * 2048 + j * 512: ki * 2048 + (j + 1) * 512],
                            start=(ki == 0), stop=(ki == 3))
                    nc.vector.tensor_copy(
                        ig_sb[:, mt * 2048 + j * 512: mt * 2048 + (j + 1) * 512],
                        pig[:, :])

            Ws = [W0s, W1s]
            bs = [b0s, b1s]
            own = [dec0T, decT]

            def lstm_step(layer, t):
                ps = ppg.tile([128, 512], f32, tag=f"psg{layer}",
                              name=f"psg{layer}_{t}")
                pT = psT[layer]
                if layer == 0:
                    mt, qb, qo = t // 16, 32 * ((t % 16) // 4), 8 * (t % 4)
                    rounds = ([("ig", 0)] + [("rec", k) for k in range(4)]
                              + [("bias", 0)])
                else:
                    rounds = ([("x", r) for r in range(4)]
                              + [("rec", k) for k in range(4)] + [("bias", 0)])
                last = len(rounds) - 1
                for r, (kind, k) in enumerate(rounds):
                    for j in range(4):
                        row = 0
                        if kind == "ig":
                            row = qb
                            lhs = id8b[qb:qb + 32, qo:qo + 8]
                            rhs = ig_sb[qb:qb + 32,
                                        mt * 2048 + j * 512:
                                        mt * 2048 + (j + 1) * 512]
                        elif kind == "x":
                            lhs = dec0T[:, 32 * (t + 1) + 8 * k:
                                        32 * (t + 1) + 8 * k + 8]
                            rhs = Ws[1][:, k * 2048 + j * 512:
                                       k * 2048 + (j + 1) * 512]
                        elif kind == "rec":
                            lhs = own[layer][:, 32 * t + 8 * k:
                                             32 * t + 8 * k + 8]
                            rhs = Ws[layer][:, (4 + k) * 2048 + j * 512:
                                            (4 + k) * 2048 + (j + 1) * 512]
                        else:
                            lhs = ones[0:1, 0:8]
                            rhs = bs[layer][0:1, j * 512:(j + 1) * 512]
                        nc.tensor.matmul(
                            ps[32 * j:32 * j + 8, :], lhs, rhs,
                            start=(r == 0), stop=(r == last),
                            tile_position=(row, 32 * j))
                sg = wp.tile([128, 512], f32, tag=f"sg{layer}", bufs=1)
                nc.scalar.activation(sg[0:104, :], ps[0:104, :], AF.Sigmoid)
                cs = c_sb[0:104, layer * 128:(layer + 1) * 128]
                tg = wp.tile([128, 128], f32, tag=f"tg{layer}")
                nc.vector.tensor_scalar(tg[0:104, :], sg[0:104, 256:384],
                                        2.0, -1.0, OP.mult, OP.add)
                m2 = wp.tile([128, 128], f32, tag=f"m2{layer}")
                nc.vector.tensor_mul(m2[0:104, :], sg[0:104, 0:128], tg[0:104, :])
                m1 = wp.tile([128, 128], f32, tag=f"m1{layer}")
                nc.vector.tensor_mul(m1[0:104, :], sg[0:104, 128:256], cs)
                nc.vector.tensor_add(cs, m1[0:104, :], m2[0:104, :])
                sc2 = wp.tile([128, 128], f32, tag=f"sc{layer}")
                nc.scalar.activation(sc2[0:104, :], cs, AF.Sigmoid, scale=2.0)
                tcc = wp.tile([128, 128], f32, tag=f"tc{layer}")
                nc.vector.tensor_scalar(tcc[0:104, :], sc2[0:104, :],
                                        2.0, -1.0, OP.mult, OP.add)
                hsp = wp.tile([128, 128], f32, tag=f"hs{layer}")
                nc.vector.tensor_mul(hsp[0:104, :], sg[0:104, 384:512],
                                     tcc[0:104, :])
                hfl = wp.tile([8, 512], f32, tag=f"hf{layer}")
                for j in range(4):
                    nc.vector.tensor_copy(hfl[0:8, 128 * j:128 * (j + 1)],
                                          hsp[32 * j:32 * j + 8, 0:128])
                for k in range(4):
                    nc.tensor.transpose(pT[:, 8 * k:8 * k + 8],
                                        hfl[0:8, 128 * k:128 * (k + 1)],
                                        id8s[0:8, :])
                nc.vector.tensor_copy(
                    own[layer][:, 32 * (t + 1):32 * (t + 2)], pT[:, 0:32])

            for t in range(T):
                lstm_step(0, t)
                if t >= 1:
                    lstm_step(1, t - 1)
            lstm_step(1, T - 1)

            decv = decT.rearrange("p (s c) -> p s c", c=32)
            projT = cp.tile([128, 4 * 512], bf16, tag="projT")
            for m in range(4):
                for r in range(4):
                    nc.vector.tensor_copy(
                        projT[:, m * 512 + r * 128: m * 512 + (r + 1) * 128],
                        decv[:, 1 + 16 * m: 17 + 16 * m, 8 * r: 8 * r + 8])
            for m in range(4):
                for n in range(8):
                    pso = pp2.tile([128, NSLICE], f32, tag="po")
                    for r in range(9):
                        if r < 4:
                            lhs = projT[:, m * 512 + r * 128:
                                        m * 512 + (r + 1) * 128]
                        elif r < 8:
                            lhs = ctxs[:, (r - 4) * 128:(r - 3) * 128]
                        else:
                            lhs = ones[0:1, 0:128]
                        if r < 8:
                            rhs = Wos[:, r * VS + n * NSLICE:
                                      r * VS + (n + 1) * NSLICE]
                        else:
                            rhs = bos[0:1, n * NSLICE:(n + 1) * NSLICE]
                        nc.tensor.matmul(pso[:, :], lhs, rhs,
                                         start=(r == 0), stop=(r == 8))
                    ob = wp.tile([128, NSLICE], f32, tag="ob")
                    nc.vector.tensor_copy(ob[:], pso[:])
                    nc.sync.dma_start(
                        out_d[128 * m:128 * (m + 1),
                              n * NSLICE:(n + 1) * NSLICE], ob[:])
    nc.finalize()
    return nc


_NC_CACHE = None


def _get_nc():
    global _NC_CACHE
    if _NC_CACHE is None:
        _NC_CACHE = _build_nc()
    return _NC_CACHE


def _host_inputs(input_ids, enc_output, h0, c0, emb, Wih0, Whh0, bih0, bhh0,
                 Wih1, Whh1, bih1, bhh1, W1, b1, W2, b2, w3, b3, Wout, bout):
    f32 = np.float32
    x = np.asarray(emb, f32)[np.asarray(input_ids).astype(np.int64)]  # [B,T,E]
    xr = x.transpose(2, 1, 0).reshape(4, 128, T, B)       # [e,p,t,b]
    xT = xr.transpose(1, 0, 2, 3).reshape(128, 4 * 512)

    def h0T(hl):
        return hl.T.reshape(4, 128, 8).transpose(1, 0, 2).reshape(128, 32)

    c_sp = np.zeros((128, 256), f32)
    for layer in range(2):
        for j in range(4):
            c_sp[32 * j:32 * j + 8, layer * 128:(layer + 1) * 128] = \
                np.asarray(c0, f32)[layer, :, 128 * j:128 * (j + 1)]

    # collapsed attention (exact in real arithmetic; see module docstring)
    u = np.asarray(W2, f32).T @ np.asarray(w3, f32)[0]
    ue = np.asarray(W1, f32)[:, :H].T @ u
    sc = np.asarray(enc_output, f32) @ ue                  # [B,S]
    sc = sc - sc.max(-1, keepdims=True)
    a = np.exp(sc)
    a /= a.sum(-1, keepdims=True)
    ctxh = np.einsum('bs,bsh->bh', a, np.asarray(enc_output, f32))  # [B,H]
    cT = ctxh.T.reshape(4, 128, 8)                         # [k,p,b]
    ctxT = np.concatenate(
        [np.tile(cT[k][:, None, :], (1, 16, 1)).reshape(128, 128)
         for k in range(4)], axis=1)                       # [128, 512]

    base = {
        "xT": xT.astype(_BF16),
        "h0a": h0T(np.asarray(h0, f32)[0]).astype(_BF16),
        "h0b": h0T(np.asarray(h0, f32)[1]).astype(_BF16),
        "c0": c_sp,
        "W0": _reorder_w(np.asarray(Wih0, f32), np.asarray(Whh0, f32)).astype(_BF16),
        "W1": _reorder_w(np.asarray(Wih1, f32), np.asarray(Whh1, f32)).astype(_BF16),
        "b0": _reorder_b(np.asarray(bih0, f32), np.asarray(bhh0, f32)).astype(_BF16),
        "b1": _reorder_b(np.asarray(bih1, f32), np.asarray(bhh1, f32)).astype(_BF16),
        "ctxT": ctxT.astype(_BF16),
        "id8": _id104(),
        "id8b": np.tile(_sel32(), (4, 1)).astype(_BF16),
        "ones": np.ones((1, 128), f32).astype(_BF16),
    }
    Wo_full = np.asarray(Wout, f32)                        # [V, 2H]
    bo_full = np.asarray(bout, f32)
    maps = []
    for k in range(NCORES):
        sh = Wo_full[k * VS:(k + 1) * VS]                  # [VS, 1024]
        Wo = np.concatenate([sh[:, 128 * r:128 * (r + 1)].T
                             for r in range(8)], axis=1)   # [128, 8*VS]
        m = dict(base)
        m["Wo"] = np.ascontiguousarray(Wo).astype(_BF16)
        m["bo"] = bo_full[k * VS:(k + 1) * VS].reshape(1, VS).astype(_BF16)
        maps.append(m)
    return maps


def kernel(**inputs):
    from concourse.bass_utils import run_bass_kernel_spmd
    nc = _get_nc()
    maps = _host_inputs(**inputs)
    res = run_bass_kernel_spmd(nc, maps, list(range(NCORES))).results
    full = np.zeros((B, T, V), np.float32)
    for k in range(NCORES):
        o = np.asarray(res[k]["out"], np.float32).reshape(T, B, VS)
        full[:, :, k * VS:(k + 1) * VS] = o.transpose(1, 0, 2)
    return full


# revision 15
# speedup vs baseline: 5364.8111x; 1.0564x over previous
"""AttnDecoder kernel for 8 trn2 NeuronCores.

Math: the reference's additive attention has NO nonlinearity between W1/W2/w3,
so scores[b,t,s] = enc[b,s]@ue + dec[b,t]@ud + const. Softmax over s cancels
the t-dependent terms exactly -> attn (and ctx) are t-independent:
    ue  = W1[:, :H].T @ (W2.T @ w3[0])
    attn[b, :] = softmax(enc[b] @ ue);  ctx[b] = attn[b] @ enc[b]
Device work = 2-layer LSTM (replicated on all 8 cores, batch-in-M col-tiled
matmuls) + vocab-sharded output projection (4000 cols/core). No collectives.
"""

import numpy as np
import ml_dtypes

B, T, S = 8, 64, 128
V, E, H = 32000, 512, 512
NCORES = 8
VS = V // NCORES  # 4000 vocab columns per core
NSLICE = 500      # psum-bank-sized N chunk for the projection

_BF16 = ml_dtypes.bfloat16


def _reorder_w(Wih, Whh):
    """[128, 8*2048] rhs layout: rounds 0-3 = Wih K-chunks, 4-7 = Whh K-chunks.
    col j*512 + g*128 + x  <-  W[g*512 + 128j + x, 128*ki + p]; g-gate rows x2
    (tanh(z) = 2*sigmoid(2z) - 1 lets one Sigmoid call cover all gates)."""
    out = np.zeros((128, 8 * 2048), np.float32)
    for r in range(8):
        Wsrc = Wih if r < 4 else Whh
        ki = r % 4
        blk = Wsrc[:, 128 * ki:128 * (ki + 1)]          # [2048, 128] (gates, p)
        t_ = blk.reshape(4, 4, 128, 128)                # [g, j, x, p]
        t_ = t_.transpose(3, 1, 0, 2)                   # [p, j, g, x]
        out[:, r * 2048:(r + 1) * 2048] = t_.reshape(128, 2048)
    w5 = out.reshape(128, 8, 4, 4, 128)                 # [p, r, j, g, x]
    w5[:, :, :, 2, :] *= 2.0
    return out


def _sel32():
    m = np.zeros((32, 32), np.float32)
    for q in range(4):
        m[8 * q:8 * (q + 1), 8 * q:8 * (q + 1)] = np.eye(8, dtype=np.float32)
    return m


def _id104():
    m = np.zeros((104, 8), np.float32)
    for j in range(4):
        m[32 * j:32 * j + 8, :] = np.eye(8, dtype=np.float32)
    return m


def _reorder_b(bih, bhh):
    bb = (bih + bhh).astype(np.float32).reshape(4, 4, 128)  # [g, j, x]
    bb = bb.transpose(1, 0, 2).copy()                       # [j, g, x]
    bb[:, 2, :] *= 2.0
    return bb.reshape(1, 2048)


def _build_nc():
    import concourse.bass as bass
    import concourse.bacc as bacc
    import concourse.mybir as mybir
    import concourse.tile as tile

    f32 = mybir.dt.float32
    bf16 = mybir.dt.bfloat16
    AF = mybir.ActivationFunctionType
    OP = mybir.AluOpType

    nc = bacc.Bacc(None, target_bir_lowering=False)
    d = {}
    d["xT"] = nc.dram_tensor("xT", [128, 4 * 512], bf16, kind="ExternalInput")
    d["h0a"] = nc.dram_tensor("h0a", [128, 32], bf16, kind="ExternalInput")
    d["h0b"] = nc.dram_tensor("h0b", [128, 32], bf16, kind="ExternalInput")
    d["c0"] = nc.dram_tensor("c0", [128, 256], f32, kind="ExternalInput")
    d["W0"] = nc.dram_tensor("W0", [128, 8 * 2048], bf16, kind="ExternalInput")
    d["W1"] = nc.dram_tensor("W1", [128, 8 * 2048], bf16, kind="ExternalInput")
    d["b0"] = nc.dram_tensor("b0", [1, 2048], bf16, kind="ExternalInput")
    d["b1"] = nc.dram_tensor("b1", [1, 2048], bf16, kind="ExternalInput")
    d["Wo"] = nc.dram_tensor("Wo", [128, 8 * VS], bf16, kind="ExternalInput")
    d["bo"] = nc.dram_tensor("bo", [1, VS], bf16, kind="ExternalInput")
    d["ctxT"] = nc.dram_tensor("ctxT", [128, 4 * 128], bf16, kind="ExternalInput")
    d["id8"] = nc.dram_tensor("id8", [104, 8], f32, kind="ExternalInput")
    d["ones"] = nc.dram_tensor("ones", [1, 128], bf16, kind="ExternalInput")
    d["id8b"] = nc.dram_tensor("id8b", [128, 32], bf16, kind="ExternalInput")
    out_d = nc.dram_tensor("out", [512, VS], f32, kind="ExternalOutput")

    with tile.TileContext(nc) as tc:
        with (
            tc.tile_pool(name="const", bufs=1) as cp,
            tc.tile_pool(name="work", bufs=2) as wp,
            tc.tile_pool(name="ps", bufs=1, space="PSUM") as pp,
            tc.tile_pool(name="psg", bufs=2, space="PSUM") as ppg,
            tc.tile_pool(name="ps2", bufs=2, space="PSUM") as pp2,
        ):
            W0s = cp.tile([128, 8 * 2048], bf16, tag="W0s")
            W1s = cp.tile([128, 8 * 2048], bf16, tag="W1s")
            Wos = cp.tile([128, 8 * VS], bf16, tag="Wos")
            xTs = cp.tile([128, 4 * 512], bf16, tag="xTs")
            dec0T = cp.tile([128, 65 * 32], bf16, tag="dec0T")
            decT = cp.tile([128, 65 * 32], bf16, tag="decT")
            c_sb = cp.tile([128, 256], f32, tag="c_sb")
            b0s = cp.tile([1, 2048], bf16, tag="b0s")
            b1s = cp.tile([1, 2048], bf16, tag="b1s")
            bos = cp.tile([1, VS], bf16, tag="bos")
            ctxs = cp.tile([128, 4 * 128], bf16, tag="ctxs")
            id8s = cp.tile([104, 8], f32, tag="id8s")
            ones = cp.tile([1, 128], bf16, tag="ones")
            id8b = cp.tile([128, 32], bf16, tag="id8b")

            # L0 weights first (needed immediately), big tensors chunked
            for r in range(8):
                nc.sync.dma_start(W0s[:, r * 2048:(r + 1) * 2048],
                                  d["W0"][:, r * 2048:(r + 1) * 2048])
            nc.sync.dma_start(xTs[:], d["xT"][:])
            nc.sync.dma_start(dec0T[:, 0:32], d["h0a"][:])
            nc.sync.dma_start(decT[:, 0:32], d["h0b"][:])
            nc.sync.dma_start(c_sb[:], d["c0"][:])
            nc.sync.dma_start(b0s[:], d["b0"][:])
            nc.sync.dma_start(b1s[:], d["b1"][:])
            nc.sync.dma_start(id8s[:], d["id8"][:])
            nc.sync.dma_start(ones[:], d["ones"][:])
            nc.sync.dma_start(id8b[:], d["id8b"][:])
            for r in range(8):
                nc.sync.dma_start(W1s[:, r * 2048:(r + 1) * 2048],
                                  d["W1"][:, r * 2048:(r + 1) * 2048])
            for r in range(8):
                nc.sync.dma_start(Wos[:, r * VS:(r + 1) * VS],
                                  d["Wo"][:, r * VS:(r + 1) * VS])
            nc.sync.dma_start(bos[:], d["bo"][:])
            nc.sync.dma_start(ctxs[:], d["ctxT"][:])

            psT = [pp.tile([128, 32], f32, tag="psT0", name="psT0"),
                   pp.tile([128, 32], f32, tag="psT1", name="psT1")]

            ig_sb = cp.tile([128, 4 * 2048], bf16, tag="ig_sb")
            for mt in range(4):
                for j in range(4):
                    pig = pp2.tile([128, 512], f32, tag="po", name=f"pig{mt}_{j}")
                    for ki in range(4):
                        nc.tensor.matmul(
                            pig[:, :],
                            xTs[:, ki * 512 + mt * 128: ki * 512 + (mt + 1) * 128],
                            W0s[:, ki * 2048 + j * 512: ki * 2048 + (j + 1) * 512],
                            start=(ki == 0), stop=False)
                    nc.tensor.matmul(
                        pig[:, :], ones[0:1, 0:128],
                        b0s[0:1, j * 512:(j + 1) * 512],
                        start=False, stop=True)
                    nc.vector.tensor_copy(
                        ig_sb[:, mt * 2048 + j * 512: mt * 2048 + (j + 1) * 512],
                        pig[:, :])

            Ws = [W0s, W1s]
            bs = [b0s, b1s]
            own = [dec0T, decT]

            def lstm_step(layer, t):
                ps = ppg.tile([128, 512], f32, tag=f"psg{layer}",
                              name=f"psg{layer}_{t}")
                pT = psT[layer]
                if layer == 0:
                    mt, qb, qo = t // 16, 32 * ((t % 16) // 4), 8 * (t % 4)
                    rounds = [("ig", 0)] + [("rec", k) for k in range(4)]
                else:
                    rounds = ([("x", r) for r in range(4)]
                              + [("rec", k) for k in range(4)] + [("bias", 0)])
                last = len(rounds) - 1
                for r, (kind, k) in enumerate(rounds):
                    for j in range(4):
                        row = 0
                        if kind == "ig":
                            row = qb
                            lhs = id8b[qb:qb + 32, qo:qo + 8]
                            rhs = ig_sb[qb:qb + 32,
                                        mt * 2048 + j * 512:
                                        mt * 2048 + (j + 1) * 512]
                        elif kind == "x":
                            lhs = dec0T[:, 32 * (t + 1) + 8 * k:
                                        32 * (t + 1) + 8 * k + 8]
                            rhs = Ws[1][:, k * 2048 + j * 512:
                                       k * 2048 + (j + 1) * 512]
                        elif kind == "rec":
                            lhs = own[layer][:, 32 * t + 8 * k:
                                             32 * t + 8 * k + 8]
                            rhs = Ws[layer][:, (4 + k) * 2048 + j * 512:
                                            (4 + k) * 2048 + (j + 1) * 512]
                        else:
                            lhs = ones[0:1, 0:8]
                            rhs = bs[layer][0:1, j * 512:(j + 1) * 512]
                        nc.tensor.matmul(
                            ps[32 * j:32 * j + 8, :], lhs, rhs,
                            start=(r == 0), stop=(r == last),
                            tile_position=(row, 32 * j))
                sg = wp.tile([128, 512], f32, tag=f"sg{layer}", bufs=1)
                nc.scalar.activation(sg[0:104, :], ps[0:104, :], AF.Sigmoid)
                cs = c_sb[0:104, layer * 128:(layer + 1) * 128]
                tg = wp.tile([128, 128], f32, tag=f"tg{layer}")
                nc.vector.tensor_scalar(tg[0:104, :], sg[0:104, 256:384],
                                        2.0, -1.0, OP.mult, OP.add)
                m2 = wp.tile([128, 128], f32, tag=f"m2{layer}")
                nc.vector.tensor_mul(m2[0:104, :], sg[0:104, 0:128], tg[0:104, :])
                m1 = wp.tile([128, 128], f32, tag=f"m1{layer}")
                nc.vector.tensor_mul(m1[0:104, :], sg[0:104, 128:256], cs)
                nc.vector.tensor_add(cs, m1[0:104, :], m2[0:104, :])
                sc2 = wp.tile([128, 128], f32, tag=f"sc{layer}")
                nc.scalar.activation(sc2[0:104, :], cs, AF.Sigmoid, scale=2.0)
                tcc = wp.tile([128, 128], f32, tag=f"tc{layer}")
                nc.vector.tensor_scalar(tcc[0:104, :], sc2[0:104, :],
                                        2.0, -1.0, OP.mult, OP.add)
                hsp = wp.tile([128, 128], f32, tag=f"hs{layer}")
                nc.vector.tensor_mul(hsp[0:104, :], sg[0:104, 384:512],
                                     tcc[0:104, :])
                hfl = wp.tile([8, 512], f32, tag=f"hf{layer}")
                for j in range(4):
                    nc.vector.tensor_copy(hfl[0:8, 128 * j:128 * (j + 1)],
                                          hsp[32 * j:32 * j + 8, 0:128])
                for k in range(4):
                    nc.tensor.transpose(pT[:, 8 * k:8 * k + 8],
                                        hfl[0:8, 128 * k:128 * (k + 1)],
                                        id8s[0:8, :])
                nc.vector.tensor_copy(
                    own[layer][:, 32 * (t + 1):32 * (t + 2)], pT[:, 0:32])

            for t in range(T):
                lstm_step(0, t)
                if t >= 1:
                    lstm_step(1, t - 1)
            lstm_step(1, T - 1)

            decv = decT.rearrange("p (s c) -> p s c", c=32)
            projT = cp.tile([128, 4 * 512], bf16, tag="projT")
            for m in range(4):
                for r in range(4):
                    nc.vector.tensor_copy(
                        projT[:, m * 512 + r * 128: m * 512 + (r + 1) * 128],
                        decv[:, 1 + 16 * m: 17 + 16 * m, 8 * r: 8 * r + 8])
            for m in range(4):
                for n in range(8):
                    pso = pp2.tile([128, NSLICE], f32, tag="po")
                    for r in range(9):
                        if r < 4:
                            lhs = projT[:, m * 512 + r * 128:
                                        m * 512 + (r + 1) * 128]
                        elif r < 8:
                            lhs = ctxs[:, (r - 4) * 128:(r - 3) * 128]
                        else:
                            lhs = ones[0:1, 0:128]
                        if r < 8:
                            rhs = Wos[:, r * VS + n * NSLICE:
                                      r * VS + (n + 1) * NSLICE]
                        else:
                            rhs = bos[0:1, n * NSLICE:(n + 1) * NSLICE]
                        nc.tensor.matmul(pso[:, :], lhs, rhs,
                                         start=(r == 0), stop=(r == 8))
                    ob = wp.tile([128, NSLICE], f32, tag="ob")
                    nc.vector.tensor_copy(ob[:], pso[:])
                    nc.sync.dma_start(
                        out_d[128 * m:128 * (m + 1),
                              n * NSLICE:(n + 1) * NSLICE], ob[:])
    nc.finalize()
    return nc


_NC_CACHE = None


def _get_nc():
    global _NC_CACHE
    if _NC_CACHE is None:
        _NC_CACHE = _build_nc()
    return _NC_CACHE


def _host_inputs(input_ids, enc_output, h0, c0, emb, Wih0, Whh0, bih0, bhh0,
                 Wih1, Whh1, bih1, bhh1, W1, b1, W2, b2, w3, b3, Wout, bout):
    f32 = np.float32
    x = np.asarray(emb, f32)[np.asarray(input_ids).astype(np.int64)]  # [B,T,E]
    xr = x.transpose(2, 1, 0).reshape(4, 128, T, B)       # [e,p,t,b]
    xT = xr.transpose(1, 0, 2, 3).reshape(128, 4 * 512)

    def h0T(hl):
        return hl.T.reshape(4, 128, 8).transpose(1, 0, 2).reshape(128, 32)

    c_sp = np.zeros((128, 256), f32)
    for layer in range(2):
        for j in range(4):
            c_sp[32 * j:32 * j + 8, layer * 128:(layer + 1) * 128] = \
                np.asarray(c0, f32)[layer, :, 128 * j:128 * (j + 1)]

    # collapsed attention (exact in real arithmetic; see module docstring)
    u = np.asarray(W2, f32).T @ np.asarray(w3, f32)[0]
    ue = np.asarray(W1, f32)[:, :H].T @ u
    sc = np.asarray(enc_output, f32) @ ue                  # [B,S]
    sc = sc - sc.max(-1, keepdims=True)
    a = np.exp(sc)
    a /= a.sum(-1, keepdims=True)
    ctxh = np.einsum('bs,bsh->bh', a, np.asarray(enc_output, f32))  # [B,H]
    cT = ctxh.T.reshape(4, 128, 8)                         # [k,p,b]
    ctxT = np.concatenate(
        [np.tile(cT[k][:, None, :], (1, 16, 1)).reshape(128, 128)
         for k in range(4)], axis=1)                       # [128, 512]

    base = {
        "xT": xT.astype(_BF16),
        "h0a": h0T(np.asarray(h0, f32)[0]).astype(_BF16),
        "h0b": h0T(np.asarray(h0, f32)[1]).astype(_BF16),
        "c0": c_sp,
        "W0": _reorder_w(np.asarray(Wih0, f32), np.asarray(Whh0, f32)).astype(_BF16),
        "W1": _reorder_w(np.asarray(Wih1, f32), np.asarray(Whh1, f32)).astype(_BF16),
        "b0": _reorder_b(np.asarray(bih0, f32), np.asarray(bhh0, f32)).astype(_BF16),
        "b1": _reorder_b(np.asarray(bih1, f32), np.asarray(bhh1, f32)).astype(_BF16),
        "ctxT": ctxT.astype(_BF16),
        "id8": _id104(),
        "id8b": np.tile(_sel32(), (4, 1)).astype(_BF16),
        "ones": np.ones((1, 128), f32).astype(_BF16),
    }
    Wo_full = np.asarray(Wout, f32)                        # [V, 2H]
    bo_full = np.asarray(bout, f32)
    maps = []
    for k in range(NCORES):
        sh = Wo_full[k * VS:(k + 1) * VS]                  # [VS, 1024]
        Wo = np.concatenate([sh[:, 128 * r:128 * (r + 1)].T
                             for r in range(8)], axis=1)   # [128, 8*VS]
        m = dict(base)
        m["Wo"] = np.ascontiguousarray(Wo).astype(_BF16)
        m["bo"] = bo_full[k * VS:(k + 1) * VS].reshape(1, VS).astype(_BF16)
        maps.append(m)
    return maps


def kernel(**inputs):
    from concourse.bass_utils import run_bass_kernel_spmd
    nc = _get_nc()
    maps = _host_inputs(**inputs)
    res = run_bass_kernel_spmd(nc, maps, list(range(NCORES))).results
    full = np.zeros((B, T, V), np.float32)
    for k in range(NCORES):
        o = np.asarray(res[k]["out"], np.float32).reshape(T, B, VS)
        full[:, :, k * VS:(k + 1) * VS] = o.transpose(1, 0, 2)
    return full
